# revision 14
# baseline (speedup 1.0000x reference)
"""Trainium2 Bass kernel for nn_CrossAttention (B=4, N=M=1024, C=768, H=12, D=64).

Sharding: pure data-parallel over 8 cores. Core c handles batch b = c // 2 and
query rows [512*(c%2), 512*(c%2)+512). Each core computes K/V for its batch
(duplicated across the 2 cores sharing a batch) so no collectives are needed.

All host-side preprocessing (sharding + transposes) is done in numpy so the
device kernel performs zero layout transposes:
  - xT  [768, 512]   = x[b, n0:n0+512, :].T        (c-major for Q projection)
  - yT  [768, 1024]  = y[b].T                      (c-major for K/V projection)
  - wqT/wkT/wvT/wpT [768, 768] = W.T               (c-major weights)
  - yw  [1, 1024], bp [1, 768]

Device dataflow (all matmuls in float32r: full PE rate at fp32 storage):
  QT[co,n]  = sum_c wqT[c,co] xT[c,n]              (feature-major Q)
  KT[co,m]  = sum_c wkT[c,co] yT[c,m] + yw[m]      (bias via rank-1 ones matmul)
  V[m,cv]   = sum_c yT[c,m] wvT[c,cv]              (sequence-major V, stored with a
                                                    ones column per head: 65-col strides)
  per head h, per m-chunk: ST[m,n] = KT_h[:,m]^T @ QT_h  -> exp(SCALE*ST) on ACT
  O'[d+1,n] accum += V_h[m, d|1]^T @ expST[m,n]    (row 64 = softmax denominator Z)
  OT[d,n]   = O'[0:64] * (1/Z) broadcast           (rank-1 ones matmul broadcast)
  out[n,co] = sum_ci OT[ci,n] wpT[ci,co] + bp[co]  (bias via rank-1 ones matmul)
"""

import sys

for _p in ("/opt/trn_rl_repo",):
    if _p not in sys.path:
        sys.path.insert(0, _p)

import numpy as np
from contextlib import ExitStack

import concourse.bass as bass
import concourse.mybir as mybir
import concourse.tile as tile
from concourse import bacc

F32 = mybir.dt.float32
F32R = mybir.dt.float32r

B = 4
N = 1024
M = 1024
C = 768
H = 12
D = 64
NSH = 512            # query rows per core
CK = C // 128        # 6 chunks of the feature dim
MK = M // 128        # 8 chunks of the key dim
SCALE = D ** -0.5
N_CORES = 8


def build_bass():
    nc = bacc.Bacc("TRN2", target_bir_lowering=False, debug=False)

    xT = nc.dram_tensor("xT", [C, NSH], F32R, kind="ExternalInput").ap()
    yT = nc.dram_tensor("yT", [C, M], F32R, kind="ExternalInput").ap()
    yw = nc.dram_tensor("yw", [1, M], F32R, kind="ExternalInput").ap()
    wqT = nc.dram_tensor("wqT", [C, C], F32R, kind="ExternalInput").ap()
    wkT = nc.dram_tensor("wkT", [C, C], F32R, kind="ExternalInput").ap()
    wvT = nc.dram_tensor("wvT", [C, C], F32R, kind="ExternalInput").ap()
    wpT = nc.dram_tensor("wpT", [C, C], F32R, kind="ExternalInput").ap()
    bp = nc.dram_tensor("bp", [1, C], F32R, kind="ExternalInput").ap()
    ones_in = nc.dram_tensor("ones_in", [1, 128], F32R, kind="ExternalInput").ap()
    out = nc.dram_tensor("out", [NSH, C], F32, kind="ExternalOutput").ap()

    wqT_c = wqT.rearrange("(k p) n -> k p n", p=128)
    wkT_c = wkT.rearrange("(k p) n -> k p n", p=128)
    wvT_c = wvT.rearrange("(k p) n -> k p n", p=128)
    wpT_c = wpT.rearrange("(k p) n -> k p n", p=128)
    xT_c = xT.rearrange("(k p) n -> k p n", p=128)
    yT_c = yT.rearrange("(k p) n -> k p n", p=128)
    out_c = out.rearrange("(k p) n -> k p n", p=128)

    with tile.TileContext(nc) as tc, ExitStack() as ctx:
        wpool = ctx.enter_context(tc.tile_pool(name="w", bufs=18))
        xpool = ctx.enter_context(tc.tile_pool(name="xt", bufs=CK))
        ypool = ctx.enter_context(tc.tile_pool(name="yt", bufs=CK))
        qpool = ctx.enter_context(tc.tile_pool(name="qt", bufs=CK))
        kpool = ctx.enter_context(tc.tile_pool(name="kt", bufs=CK))
        vpool = ctx.enter_context(tc.tile_pool(name="vs", bufs=MK))
        opool = ctx.enter_context(tc.tile_pool(name="ot", bufs=CK))
        epool = ctx.enter_context(tc.tile_pool(name="es", bufs=4))
        outpool = ctx.enter_context(tc.tile_pool(name="outs", bufs=2))
        cpool = ctx.enter_context(tc.tile_pool(name="const", bufs=1))
        zpool = ctx.enter_context(tc.tile_pool(name="z", bufs=2))
        ppool = ctx.enter_context(tc.tile_pool(name="pp", bufs=2, space="PSUM"))
        spool = ctx.enter_context(tc.tile_pool(name="sp", bufs=2, space="PSUM"))
        opsum = ctx.enter_context(tc.tile_pool(name="op", bufs=2, space="PSUM"))
        zbpool = ctx.enter_context(tc.tile_pool(name="zb", bufs=1, space="PSUM"))

        # ---- constants / small inputs ----
        ones = cpool.tile([1, 128], F32R, tag="ones")
        nc.sync.dma_start(out=ones, in_=ones_in)
        yw_s = cpool.tile([1, M], F32R, tag="yws")
        nc.sync.dma_start(out=yw_s, in_=yw)
        bp_s = cpool.tile([1, C], F32R, tag="bps")
        nc.sync.dma_start(out=bp_s, in_=bp)

        # ---- weight / activation loads (emission order = priority order) ----
        wq = []
        xt = []
        for i in range(CK):
            t = wpool.tile([128, C], F32R, tag="w")
            nc.sync.dma_start(out=t, in_=wqT_c[i])
            wq.append(t)
            t = xpool.tile([128, NSH], F32R, tag="xt")
            nc.sync.dma_start(out=t, in_=xT_c[i])
            xt.append(t)
        wk = []
        yt = []
        for i in range(CK):
            t = wpool.tile([128, C], F32R, tag="w")
            nc.sync.dma_start(out=t, in_=wkT_c[i])
            wk.append(t)
            t = ypool.tile([128, M], F32R, tag="yt")
            nc.sync.dma_start(out=t, in_=yT_c[i])
            yt.append(t)
        wv = []
        for i in range(CK):
            t = wpool.tile([128, C], F32R, tag="w")
            nc.sync.dma_start(out=t, in_=wvT_c[i])
            wv.append(t)
        wp = []
        for i in range(CK):
            t = wpool.tile([128, C], F32R, tag="w")  # reuses the 6 wq slots
            nc.sync.dma_start(out=t, in_=wpT_c[i])
            wp.append(t)

        # ---- Q projection: QT[co*128:.., n] ----
        qt = []
        for co in range(CK):
            ps = ppool.tile([128, 512], F32, tag="pp")
            for ci in range(CK):
                nc.tensor.matmul(
                    ps,
                    wq[ci][:, co * 128:(co + 1) * 128],
                    xt[ci],
                    start=(ci == 0),
                    stop=(ci == CK - 1),
                )
            t = qpool.tile([128, NSH], F32R, tag="qt")
            nc.scalar.copy(t, ps)
            qt.append(t)

        # ---- K projection + yw bias: KT[co*128:.., m] ----
        kt = []
        for co in range(CK):
            t = kpool.tile([128, M], F32R, tag="kt")
            kt.append(t)
            for mh in range(2):
                ps = ppool.tile([128, 512], F32, tag="pp")
                for ci in range(CK):
                    nc.tensor.matmul(
                        ps,
                        wk[ci][:, co * 128:(co + 1) * 128],
                        yt[ci][:, mh * 512:(mh + 1) * 512],
                        start=(ci == 0),
                        stop=False,
                    )
                # += ones^T @ yw  (broadcast yw over the 128 co-partitions)
                nc.tensor.matmul(
                    ps,
                    ones,
                    yw_s[:, mh * 512:(mh + 1) * 512],
                    start=False,
                    stop=True,
                )
                nc.vector.tensor_copy(t[:, mh * 512:(mh + 1) * 512], ps)

        # ---- V projection (sequence-major, 65-col per-head layout) ----
        vt = []
        for mc in range(MK):
            t = vpool.tile([128, 65 * H], F32R, tag="vs")
            vt.append(t)
            t3 = t.rearrange("p (h e) -> p h e", e=65)
            # ones column per head via broadcast DMA (memset can't write f32r)
            ones_bcast = bass.AP(
                tensor=ones_in.tensor,
                offset=0,
                ap=[[0, 128], [0, H], [1, 1]],
            )
            nc.sync.dma_start(out=t3[:, :, 64:65], in_=ones_bcast)
            for nh in range(2):
                ps = ppool.tile([128, 384], F32, tag="pp")
                for ci in range(CK):
                    nc.tensor.matmul(
                        ps,
                        yt[ci][:, mc * 128:(mc + 1) * 128],
                        wv[ci][:, nh * 384:(nh + 1) * 384],
                        start=(ci == 0),
                        stop=(ci == CK - 1),
                    )
                src = ps.rearrange("p (h e) -> p h e", e=64)
                dst = t3[:, nh * 6:(nh + 1) * 6, 0:64]
                nc.vector.tensor_copy(dst, src)

        # ---- attention (streaming over m-chunks; no max subtraction) ----
        ot = [
            opool.tile([128, NSH], F32R, tag="ot", name=f"ot{i}")
            for i in range(CK)
        ]
        for h in range(H):
            ktile = kt[h // 2]
            qtile = qt[h // 2]
            pr = 64 * (h % 2)
            op = opsum.tile([128, 512], F32, tag="op")
            for mc in range(MK):
                sp = spool.tile([128, 512], F32, tag="sp")
                nc.tensor.matmul(
                    sp,
                    ktile[pr:pr + 64, mc * 128:(mc + 1) * 128],
                    qtile[pr:pr + 64, :],
                    start=True,
                    stop=True,
                )
                es = epool.tile([128, 512], F32R, tag="es")
                nc.scalar.activation(
                    es, sp, mybir.ActivationFunctionType.Exp, scale=SCALE
                )
                nc.tensor.matmul(
                    op[0:65, :],
                    vt[mc][:, 65 * h:65 * h + 65],
                    es,
                    start=(mc == 0),
                    stop=(mc == MK - 1),
                )
            zi = zpool.tile([1, 512], F32R, tag="z")
            with nc.allow_low_precision(reason="f32r is fp32-width storage"):
                nc.vector.reciprocal(zi, op[64:65, :])
            zb = zbpool.tile([64, 512], F32, tag="zb")
            nc.tensor.matmul(zb, ones[:, 0:64], zi, start=True, stop=True)
            zbs = zpool.tile([64, 512], F32, tag="zbs")
            nc.scalar.copy(zbs, zb)
            nc.vector.tensor_tensor(
                ot[h // 2][pr:pr + 64, :], op[0:64, :], zbs, mybir.AluOpType.mult
            )

        # ---- output projection + bias ----
        for n4 in range(4):
            outs = outpool.tile([128, C], F32, tag="outs")
            for nh in range(2):
                ps = ppool.tile([128, 384], F32, tag="pp")
                for ci in range(CK):
                    nc.tensor.matmul(
                        ps,
                        ot[ci][:, n4 * 128:(n4 + 1) * 128],
                        wp[ci][:, nh * 384:(nh + 1) * 384],
                        start=(ci == 0),
                        stop=False,
                    )
                nc.tensor.matmul(
                    ps,
                    ones,
                    bp_s[:, nh * 384:(nh + 1) * 384],
                    start=False,
                    stop=True,
                )
                nc.scalar.copy(outs[:, nh * 384:(nh + 1) * 384], ps)
            nc.sync.dma_start(out=out_c[n4], in_=outs)

    if not nc.is_finalized():
        nc.finalize()
    return nc


_NC_CACHE = None


def _get_nc():
    global _NC_CACHE
    if _NC_CACHE is None:
        _NC_CACHE = build_bass()
    return _NC_CACHE


def _round_f32r(a):
    """Round to the bf16+bf16 representable set the PE's fp32r path uses."""
    import ml_dtypes

    a32 = np.asarray(a, np.float32)
    hi = a32.astype(ml_dtypes.bfloat16).astype(np.float32)
    lo = (a32 - hi).astype(ml_dtypes.bfloat16).astype(np.float32)
    return hi + lo


def make_in_maps(x, y, yw, Wq, Wk, Wv, Wp, bp):
    x = _round_f32r(np.asarray(x, dtype=np.float32))
    y = _round_f32r(np.asarray(y, dtype=np.float32))
    yw = _round_f32r(np.asarray(yw, dtype=np.float32))
    wqT = _round_f32r(np.ascontiguousarray(np.asarray(Wq, dtype=np.float32).T))
    wkT = _round_f32r(np.ascontiguousarray(np.asarray(Wk, dtype=np.float32).T))
    wvT = _round_f32r(np.ascontiguousarray(np.asarray(Wv, dtype=np.float32).T))
    wpT = _round_f32r(np.ascontiguousarray(np.asarray(Wp, dtype=np.float32).T))
    bp = np.asarray(bp, dtype=np.float32).reshape(1, C)

    in_maps = []
    for c in range(N_CORES):
        b, half = divmod(c, 2)
        n0 = half * NSH
        in_maps.append(
            {
                "xT": np.ascontiguousarray(x[b, n0:n0 + NSH, :].T),
                "yT": np.ascontiguousarray(y[b].T),
                "yw": np.ascontiguousarray(yw[b].reshape(1, M)),
                "wqT": wqT,
                "wkT": wkT,
                "wvT": wvT,
                "wpT": wpT,
                "bp": bp,
                "ones_in": np.ones((1, 128), np.float32),
            }
        )
    return in_maps


def run(inputs, trace=False):
    """Returns (full_output, BassKernelResults)."""
    from concourse.bass_utils import run_bass_kernel_spmd

    nc = _get_nc()
    in_maps = make_in_maps(**inputs)
    res = run_bass_kernel_spmd(
        nc, in_maps, list(range(N_CORES)), trace=trace
    )
    full = np.empty((B, N, C), dtype=np.float32)
    for c in range(N_CORES):
        b, half = divmod(c, 2)
        n0 = half * NSH
        full[b, n0:n0 + NSH, :] = res.results[c]["out"]
    return full, res


def kernel(**inputs):
    full, _ = run(inputs, trace=False)
    return full
